# revision 22
# baseline (speedup 1.0000x reference)
"""nn_ConvSOM_dense1 Trainium2 kernel v2: 3x GCNConv + SOM scatter + dense head.

Self-contained: host prep (edge sort, degree norm, table-order permutation),
Bass/Tile SPMD kernel on 8 NeuronCores, host gather of per-core G partials +
tiny final linear.

Design (1.78ms vs 3.07ms baseline on HW):
- conv1: x is replicated (feature-major, table-row order); every core computes
  the full h1 table locally -> no AllGather for conv1; single merged pass.
- table rows are chunk-half-major so each half's rows come from a per-core
  contiguous range; halves are AllGathered separately: AG(half0) fires
  mid-pass-2 of the previous conv, AG(half1) at conv end, hidden under the
  next conv's pass-1 (two-pass aggregation with an SBUF partial +
  identity-matmul reload for convs 2-3).
- gathers: one 256B descriptor per edge (descriptor-rate bound, ~2.6-3.4
  ns/desc aggregate on 4 SWDGE queues), 2-tile groups, 8-deep gb buffering,
  within-segment source-sorted for HBM locality.
- scatter one-hot matrices (norm folded in) are STATIC -> precomputed on host
  and streamed as a bf16 input on the SP HWDGE ring (no per-edge DVE work);
  table writes/h-staging writes ride the ACT HWDGE ring to avoid FIFO
  head-of-line blocking.
- SOM: per-tile distance matmuls + argmax only; all scalar math batched over
  [128,49] tensors at the end; per-graph G accumulated in PSUM over 49
  one-hot matmuls.
"""
import dataclasses as _dc
import numpy as np
import ml_dtypes

N = 50000
E = 800000
C = 128
P0, P1 = 16, 16
NUM_GRAPHS = 64
SIGMA = 2.0
NEG_SLOPE = 0.01
NC_ = 8
NPC = 6272            # nodes per core = 49*128
NSTAR = NC_ * NPC     # 50176
NT = NPC // 128       # 49 node tiles / core
H0T, H1T = 25, 24     # tiles per half (per core)
H0SZ, H1SZ = H0T * 128, H1T * 128      # 3200, 3072
TB0 = NC_ * H0SZ      # 25600 table rows in half0
TB1 = NC_ * H1SZ      # 24576 rows in half1
TROWS = TB0 + TB1     # 50176
NQ = 4                # SWDGE queues
GRP = 2               # node tiles per grouped gather
P = 128

_CACHE = {}
TRACE = False
LAST_EXEC_NS = None
LAST_INSTS = None


def _tabrow(g):
    """Global node id -> table row (chunk-half-major)."""
    r = g // NPC
    i = g % NPC
    return np.where(i < H0SZ, r * H0SZ + i, TB0 + r * H1SZ + (i - H0SZ))


def _host_prep(x, edge_index, batch):
    src = np.asarray(edge_index[0], dtype=np.int64)
    dst = np.asarray(edge_index[1], dtype=np.int64)
    loops = np.arange(N, dtype=np.int64)
    s = np.concatenate([src, loops])
    d = np.concatenate([dst, loops])
    deg = np.bincount(d, minlength=N).astype(np.float32)
    dinv = np.where(deg > 0, deg ** -0.5, 0.0).astype(np.float32)
    norm = dinv[s] * dinv[d]

    srow = _tabrow(s)
    h = (srow >= TB0).astype(np.int64)
    idxloc = (srow - h * TB0).astype(np.int64)
    core = d // NPC
    t = (d % NPC) // 128
    dl = d % 128

    key = core * (NT * 2) + t * 2 + h
    counts = np.bincount(key, minlength=NC_ * NT * 2).reshape(NC_, NT, 2)
    T = np.maximum(np.ceil(counts.max(axis=0) / 128).astype(np.int64), 1)  # [NT,2]

    # slot space: all h0 segments (t=0..48), then all h1 segments
    off = np.zeros((NT, 2), np.int64)
    acc = 0
    for hh in range(2):
        for tt in range(NT):
            off[tt, hh] = acc
            acc += int(T[tt, hh]) * 128
    TOT = acc

    # secondary sort by source row: consecutive gather descriptors hit
    # ascending HBM addresses (row-buffer locality)
    order = np.lexsort((idxloc, key))
    sk, si, sd_l, sn = key[order], idxloc[order], dl[order], norm[order]
    grp_start = np.zeros(NC_ * NT * 2, np.int64)
    cnt_flat = counts.reshape(-1)
    grp_start[1:] = np.cumsum(cnt_flat)[:-1]
    rank = np.arange(len(sk)) - grp_start[sk]
    seg = sk % (NT * 2)
    slot = off.reshape(-1)[seg] + rank
    score_core = sk // (NT * 2)

    idx_all = np.zeros((NC_, TOT), np.int16)
    idx_all[score_core, slot] = si.astype(np.int16)
    idx16 = idx_all.reshape(NC_, TOT // 16, 16).transpose(0, 2, 1).copy()

    # prebuilt scatter matrices: ohw[core, slot%128, (slot//128)*128 + dl] = norm
    ohw = np.zeros((NC_, 128, TOT), ml_dtypes.bfloat16)
    ohw[score_core, slot % 128, (slot // 128) * 128 + sd_l] = sn

    # x in table-row order, feature-major, bf16 (replicated to all cores)
    xpad = np.zeros((NSTAR, C), np.float32)
    xpad[:N] = np.asarray(x, np.float32)
    rows = _tabrow(np.arange(NSTAR, dtype=np.int64))
    xtab = np.zeros((TROWS, C), np.float32)
    xtab[rows] = xpad
    xTt = xtab.T.astype(ml_dtypes.bfloat16).copy()    # [128, TROWS]

    bpad = np.full(NSTAR, -1.0, np.float32)
    bpad[:N] = np.asarray(batch, np.float32)
    batch16 = bpad.reshape(NC_, NT, 128).transpose(0, 2, 1).copy()  # [NC_,128,49]

    return dict(T=T, off=off, TOT=TOT, idx16=idx16, ohw=ohw,
                xTt=xTt, batch16=batch16)


_T = None
_OFF = None
_GROUPS = None
_GTMAX = None
_TOT = None


def _build():
    import concourse.bass as bass
    import concourse.bacc as bacc
    import concourse.tile as tile
    import concourse.mybir as mybir
    from concourse.library_config import mlp
    dt = mybir.dt
    AF = mybir.ActivationFunctionType
    OP = mybir.AluOpType
    INV2S2 = 1.0 / (2.0 * SIGMA * SIGMA)
    T, OFF, GROUPS, GTMAX, TOT = _T, _OFF, _GROUPS, _GTMAX, _TOT
    TSEGMAX = int(T.max())

    nc = bacc.Bacc("TRN2", target_bir_lowering=False, debug=False,
                   num_devices=NC_, num_swdge_queues=NQ)
    xTt_d = nc.dram_tensor("xTt", [P, TROWS], dt.bfloat16, kind="ExternalInput")
    idx_d = nc.dram_tensor("idx16", [16, TOT // 16], dt.int16, kind="ExternalInput")
    ohw_d = nc.dram_tensor("ohw16", [P, TOT], dt.bfloat16, kind="ExternalInput")
    bt_d = nc.dram_tensor("batch16", [P, NT], dt.float32, kind="ExternalInput")
    W_d = [nc.dram_tensor(f"W{k}", [C, C], dt.bfloat16, kind="ExternalInput")
           for k in (1, 2, 3)]
    b_d = [nc.dram_tensor(f"b{k}", [C, 1], dt.float32, kind="ExternalInput")
           for k in (1, 2, 3)]
    sft_d = nc.dram_tensor("SfT", [3 * C, 256], dt.bfloat16, kind="ExternalInput")
    srow_d = nc.dram_tensor("srow", [1, 256], dt.float32, kind="ExternalInput")
    g_out = nc.dram_tensor("g_out", [64, 256], dt.float32, kind="ExternalOutput")

    with tile.TileContext(nc) as tc:
        with (
            tc.tile_pool(name="cst", bufs=1) as cst,
            tc.tile_pool(name="wrk", bufs=1) as wrk,
            tc.tile_pool(name="xc", bufs=2) as xcp,
            tc.tile_pool(name="gb", bufs=9) as gbp,
            tc.tile_pool(name="ow", bufs=4) as owp,
            tc.tile_pool(name="sb", bufs=4) as sb,
            tc.tile_pool(name="tl", bufs=1) as tl,
            tc.tile_pool(name="ps", bufs=4, space="PSUM") as ps,
            tc.tile_pool(name="pshs", bufs=2, space="PSUM") as pshs,
            tc.tile_pool(name="psg", bufs=1, space="PSUM") as psg,
            tc.tile_pool(name="dram", bufs=1, space="DRAM") as dram,
        ):
            nc.gpsimd.load_library(mlp)
            # ---- constants ----
            idx_sb = cst.tile([128, TOT // 16], dt.int16)
            for q in range(NQ):
                nc.sync.dma_start(idx_sb[q * 32:q * 32 + 16, :], idx_d[:])
                nc.sync.dma_start(idx_sb[q * 32 + 16:q * 32 + 32, :], idx_d[:])
            btl_sb = cst.tile([P, NT], dt.float32)
            nc.sync.dma_start(btl_sb[:], bt_d[:])
            W_sb, b_sb, sft_sb = [], [], []
            for k in range(3):
                w = cst.tile([C, C], dt.bfloat16, tag=f"W{k}")
                nc.sync.dma_start(w[:], W_d[k][:])
                W_sb.append(w)
                b = cst.tile([C, 1], dt.float32, tag=f"b{k}")
                nc.sync.dma_start(b[:], b_d[k][:])
                b_sb.append(b)
                sft = cst.tile([C, 256], dt.bfloat16, tag=f"sft{k}")
                nc.sync.dma_start(sft[:], sft_d[k * C:(k + 1) * C, :])
                sft_sb.append(sft)
            srow_sb = cst.tile([1, 256], dt.float32)
            nc.sync.dma_start(srow_sb[:], srow_d[:])

            iota_i = cst.tile([P, P], dt.int32)
            nc.gpsimd.iota(iota_i[:], pattern=[[1, P]], base=0,
                           channel_multiplier=0)
            # ident[p, j] = (p == j): iota(p+j) - iota(j) = p, compare vs j
            iotap_i = cst.tile([P, P], dt.int32)
            nc.gpsimd.iota(iotap_i[:], pattern=[[1, P]], base=0,
                           channel_multiplier=1)
            iodif = cst.tile([P, P], dt.int32)
            nc.vector.tensor_tensor(out=iodif[:], in0=iotap_i[:],
                                    in1=iota_i[:], op=OP.subtract)
            ident_b = cst.tile([P, P], dt.bfloat16)
            nc.vector.tensor_tensor(out=ident_b[:], in0=iodif[:],
                                    in1=iota_i[:], op=OP.is_equal)
            iota64_i = cst.tile([P, 64], dt.int32)
            nc.gpsimd.iota(iota64_i[:], pattern=[[1, 64]], base=0,
                           channel_multiplier=0)
            iota64 = cst.tile([P, 64], dt.float32)
            nc.vector.tensor_copy(iota64[:], iota64_i[:])
            iota16_i = cst.tile([P, 16], dt.int32)
            nc.gpsimd.iota(iota16_i[:], pattern=[[1, 16]], base=0,
                           channel_multiplier=0)
            iota16 = cst.tile([P, 16], dt.float32)
            nc.vector.tensor_copy(iota16[:], iota16_i[:])
            ones_colb = cst.tile([P, 1], dt.bfloat16)
            nc.vector.memset(ones_colb[:], 1.0)
            ones_row = cst.tile([1, P], dt.float32)
            nc.vector.memset(ones_row[:], 1.0)

            # ---- persistent work tiles ----
            x_cur = [wrk.tile([P, NPC], dt.bfloat16, tag=f"x{k}", name=f"x{k}")
                     for k in range(3)]
            part = wrk.tile([P, NT, P], dt.bfloat16, tag="part", name="part")
            hsh_st = wrk.tile([P, NT, P], dt.bfloat16, tag="hsh", name="hsh")
            hs_acc = wrk.tile([P, NT], dt.float32, tag="hsacc", name="hsacc")
            mx49 = wrk.tile([P, NT], dt.float32, tag="mx49", name="mx49")
            mi49 = wrk.tile([P, NT], dt.uint32, tag="mi49", name="mi49")

            tab1_0 = dram.tile([TB0, C], dt.bfloat16, tag="tab10", name="tab10")
            tab1_1 = dram.tile([TB1, C], dt.bfloat16, tag="tab11", name="tab11")
            hn_sh = {}
            hn_f = {}
            for k in (2, 3):
                for hh in range(2):
                    sz = TB0 if hh == 0 else TB1
                    hn_sh[(k, hh)] = dram.tile([sz // NC_, C], dt.bfloat16,
                                               tag=f"hs{k}{hh}", name=f"hs{k}{hh}")
                    hn_f[(k, hh)] = dram.tile([sz, C], dt.bfloat16,
                                              tag=f"hf{k}{hh}", name=f"hf{k}{hh}",
                                              addr_space="Shared")

            def dram_3d(tensor, rowbase, ntiles):
                v = tensor[rowbase:rowbase + ntiles * 128, :]
                return _dc.replace(v, ap=[[C, 128], [128 * C, ntiles], [1, C]])

            # ---- conv1 table build: h1 = x @ W1 for ALL table rows ----
            CH = 20  # tiles per build chunk; half0 = 200 tiles = 10 chunks
            chunks = []
            base_t = 0
            while base_t < TROWS // 128:
                n = min(CH, TROWS // 128 - base_t)
                chunks.append((base_t, n))
                base_t += n
            for (tb, ntile) in chunks:
                xc = xcp.tile([P, CH * 128], dt.bfloat16, tag="xc")
                nc.scalar.dma_start(xc[:, :ntile * 128],
                                    xTt_d[:, tb * 128:(tb + ntile) * 128])
                hst = xcp.tile([P, CH, P], dt.bfloat16, tag="hst")
                for j in range(ntile):
                    h_ps = ps.tile([P, C], dt.float32, space="PSUM", tag="w")
                    nc.tensor.matmul(h_ps[:], lhsT=xc[:, j * 128:(j + 1) * 128],
                                     rhs=W_sb[0][:], start=True, stop=True)
                    if j % 2 == 0:
                        nc.vector.tensor_copy(hst[:, j, :], h_ps[:])
                    else:
                        nc.scalar.activation(hst[:, j, :], h_ps[:], AF.Identity)
                if tb < H0T * NC_:
                    nc.scalar.dma_start(dram_3d(tab1_0, tb * 128, ntile),
                                        hst[:, :ntile, :])
                else:
                    nc.scalar.dma_start(
                        dram_3d(tab1_1, (tb - H0T * NC_) * 128, ntile),
                        hst[:, :ntile, :])

            gq = 0

            def post_tile(k, t, agg):
                sl = slice(t * 128, (t + 1) * 128)
                nc.scalar.activation(x_cur[k][:, sl], agg[:], AF.Lrelu,
                                     bias=b_sb[k][:, :1], alpha=NEG_SLOPE)
                sq = sb.tile([P, P], dt.bfloat16, tag="sq")
                nc.vector.tensor_tensor(out=sq[:], in0=x_cur[k][:, sl],
                                        in1=x_cur[k][:, sl], op=OP.mult)
                hs_ps = pshs.tile([P, 1], dt.float32, space="PSUM", tag="hs")
                nc.tensor.matmul(hs_ps[:], lhsT=sq[:], rhs=ones_colb[:],
                                 start=True, stop=True, skip_group_check=True)
                if k == 0:
                    nc.vector.tensor_copy(hs_acc[:, t:t + 1], hs_ps[:])
                else:
                    nc.vector.tensor_tensor(out=hs_acc[:, t:t + 1],
                                            in0=hs_acc[:, t:t + 1],
                                            in1=hs_ps[:, :1], op=OP.add)
                if k < 2:
                    h_ps = ps.tile([P, C], dt.float32, space="PSUM", tag="w")
                    nc.tensor.matmul(h_ps[:], lhsT=x_cur[k][:, sl],
                                     rhs=W_sb[k + 1][:], start=True, stop=True,
                                     skip_group_check=True)
                    nc.vector.tensor_copy(hsh_st[:, t, :], h_ps[:])
                    lt = t if t < H0T else t - H0T
                    tgt = hn_sh[(k + 2, 0 if t < H0T else 1)]
                    nc.scalar.dma_start(tgt[lt * 128:(lt + 1) * 128, :],
                                        hsh_st[:, t, :])
                else:
                    D_ps = ps.tile([P, 256], dt.float32, space="PSUM", tag="w")
                    for kk in range(3):
                        nc.tensor.matmul(D_ps[:], lhsT=x_cur[kk][:, sl],
                                         rhs=sft_sb[kk][:], start=(kk == 0),
                                         stop=False, skip_group_check=True)
                    nc.tensor.matmul(D_ps[:], lhsT=ones_row[:], rhs=srow_sb[:],
                                     start=False, stop=True,
                                     skip_group_check=True)
                    mx = sb.tile([P, 8], dt.float32, tag="mx")
                    mi = sb.tile([P, 8], dt.uint32, tag="mi")
                    nc.vector.max_with_indices(mx[:], mi[:], D_ps[:])
                    nc.vector.tensor_copy(mx49[:, t:t + 1], mx[:, :1])
                    nc.vector.tensor_copy(mi49[:, t:t + 1], mi[:, :1])

            def emit_ag(kk, hhh):
                nc.gpsimd.collective_compute(
                    "AllGather", mybir.AluOpType.bypass,
                    replica_groups=[list(range(NC_))],
                    ins=[hn_sh[(kk, hhh)].opt()], outs=[hn_f[(kk, hhh)].opt()])

            for k in range(3):   # conv index 0,1,2
                if k == 0:
                    views = [tab1_0[:], tab1_1[:]]
                else:
                    views = [hn_f[(k + 1, 0)][:], hn_f[(k + 1, 1)][:]]

                if k == 0:
                    # merged single pass: both halves per group, no partials
                    for gi in range(len(GROUPS[0])):
                        gbo = []
                        for hh in range(2):
                            (tiles, soff, Tg) = GROUPS[hh][gi]
                            gb = gbp.tile([P, GTMAX, P], dt.bfloat16, tag="gb")
                            nc.gpsimd.dma_gather(
                                out_ap=gb[:, :Tg, :], in_ap=views[hh],
                                idxs_ap=idx_sb[:, soff // 16:
                                               (soff + Tg * 128) // 16],
                                num_idxs=Tg * 128, num_idxs_reg=Tg * 128,
                                elem_size=P, single_packet=False,
                                queue_num=gq % NQ)
                            gq += 1
                            owb = owp.tile([P, GTMAX * 128], dt.bfloat16,
                                           tag="ow")
                            nc.sync.dma_start(owb[:, :Tg * 128],
                                              ohw_d[:, soff:soff + Tg * 128])
                            gbo.append((gb, owb, soff))
                        tiles = GROUPS[0][gi][0]
                        for t in tiles:
                            agg = ps.tile([P, C], dt.float32, space="PSUM",
                                          tag="w")
                            first = True
                            for hh in range(2):
                                gb, owb, soff = gbo[hh]
                                Tt = int(T[t, hh])
                                base = (int(OFF[t, hh]) - soff) // 128
                                for tt in range(Tt):
                                    nc.tensor.matmul(
                                        agg[:], lhsT=gb[:, base + tt, :],
                                        rhs=owb[:, (base + tt) * 128:
                                                (base + tt + 1) * 128],
                                        start=first,
                                        stop=(hh == 1 and tt == Tt - 1),
                                        skip_group_check=True)
                                    first = False
                            post_tile(0, t, agg)
                        if (H0T - 1) in tiles:
                            emit_ag(2, 0)
                    emit_ag(2, 1)
                else:
                    for hh in range(2):
                        for (tiles, soff, Tg) in GROUPS[hh]:
                            gb = gbp.tile([P, GTMAX, P], dt.bfloat16, tag="gb")
                            nc.gpsimd.dma_gather(
                                out_ap=gb[:, :Tg, :], in_ap=views[hh],
                                idxs_ap=idx_sb[:, soff // 16:
                                               (soff + Tg * 128) // 16],
                                num_idxs=Tg * 128, num_idxs_reg=Tg * 128,
                                elem_size=P, single_packet=False,
                                queue_num=gq % NQ)
                            gq += 1
                            owb = owp.tile([P, GTMAX * 128], dt.bfloat16,
                                           tag="ow")
                            nc.sync.dma_start(owb[:, :Tg * 128],
                                              ohw_d[:, soff:soff + Tg * 128])
                            for t in tiles:
                                Tt = int(T[t, hh])
                                base = (int(OFF[t, hh]) - soff) // 128
                                agg = ps.tile([P, C], dt.float32, space="PSUM",
                                              tag="w")
                                if hh == 0:
                                    for tt in range(Tt):
                                        nc.tensor.matmul(
                                            agg[:], lhsT=gb[:, base + tt, :],
                                            rhs=owb[:, (base + tt) * 128:
                                                    (base + tt + 1) * 128],
                                            start=(tt == 0),
                                            stop=(tt == Tt - 1),
                                            skip_group_check=True)
                                    nc.vector.tensor_copy(part[:, t, :],
                                                          agg[:])
                                else:
                                    nc.tensor.matmul(agg[:], lhsT=ident_b[:],
                                                     rhs=part[:, t, :],
                                                     start=True, stop=False,
                                                     skip_group_check=True)
                                    for tt in range(Tt):
                                        nc.tensor.matmul(
                                            agg[:], lhsT=gb[:, base + tt, :],
                                            rhs=owb[:, (base + tt) * 128:
                                                    (base + tt + 1) * 128],
                                            start=False, stop=(tt == Tt - 1),
                                            skip_group_check=True)
                                    post_tile(k, t, agg)
                            if (k < 2 and hh == 1 and (H0T - 1) in tiles):
                                emit_ag(k + 2, 0)
                    if k < 2:
                        emit_ag(k + 2, 1)

            # ---- SOM batched tail ----
            wj_u = sb.tile([P, NT], dt.uint32, tag="wju")
            nc.vector.tensor_scalar(out=wj_u[:], in0=mi49[:], scalar1=15,
                                    scalar2=None, op0=mybir.AluOpType.bitwise_and)
            wi_u = sb.tile([P, NT], dt.uint32, tag="wiu")
            nc.vector.tensor_scalar(out=wi_u[:], in0=mi49[:], scalar1=4,
                                    scalar2=None,
                                    op0=mybir.AluOpType.logical_shift_right)
            wj_f = sb.tile([P, NT], dt.float32, tag="wjf")
            wi_f = sb.tile([P, NT], dt.float32, tag="wif")
            nc.vector.tensor_copy(wj_f[:], wj_u[:])
            nc.vector.tensor_copy(wi_f[:], wi_u[:])
            m2 = sb.tile([P, NT], dt.float32, tag="m2")
            nc.vector.tensor_scalar(out=m2[:], in0=mx49[:], scalar1=-2.0,
                                    scalar2=None, op0=mybir.AluOpType.mult)
            d2 = sb.tile([P, NT], dt.float32, tag="d2")
            nc.vector.tensor_tensor(out=d2[:], in0=m2[:], in1=hs_acc[:],
                                    op=mybir.AluOpType.add)
            nc.vector.tensor_scalar_max(d2[:], d2[:], 0.0)
            mind = sb.tile([P, NT], dt.float32, tag="mind")
            nc.scalar.activation(mind[:], d2[:],
                                 AF.Sqrt)
            hsv = sb.tile([P, NT], dt.float32, tag="hsv")
            nc.scalar.activation(hsv[:], mind[:], AF.Exp, scale=-1.0)

            def bc3(apv, tstride, tn, inner, innerstride):
                return _dc.replace(apv, ap=[apv.ap[0], [tstride, tn],
                                            [innerstride, inner]])

            dx = tl.tile([P, NT * 16], dt.float32, tag="dx")
            iot = iota16[:]
            nc.vector.tensor_tensor(
                out=dx[:], in0=_dc.replace(iot, ap=[iot.ap[0], [0, NT],
                                                    iot.ap[1]]),
                in1=bc3(wi_f[:], 1, NT, 16, 0), op=mybir.AluOpType.subtract)
            nc.vector.tensor_tensor(out=dx[:], in0=dx[:], in1=dx[:],
                                    op=mybir.AluOpType.mult)
            nc.scalar.activation(dx[:], dx[:], AF.Exp, scale=-INV2S2)
            dy = tl.tile([P, NT * 16], dt.float32, tag="dy")
            nc.vector.tensor_tensor(
                out=dy[:], in0=_dc.replace(iot, ap=[iot.ap[0], [0, NT],
                                                    iot.ap[1]]),
                in1=bc3(wj_f[:], 1, NT, 16, 0), op=mybir.AluOpType.subtract)
            nc.vector.tensor_tensor(out=dy[:], in0=dy[:], in1=dy[:],
                                    op=mybir.AluOpType.mult)
            nc.scalar.activation(dy[:], dy[:], AF.Exp, scale=-INV2S2)
            # fold hs into dx
            nc.vector.tensor_tensor(out=dx[:], in0=dx[:],
                                    in1=bc3(hsv[:], 1, NT, 16, 0),
                                    op=mybir.AluOpType.mult)

            G_ps = psg.tile([64, 256], dt.float32, space="PSUM", tag="G")
            TGRP = 7
            for g0 in range(0, NT, TGRP):
                ng = min(TGRP, NT - g0)
                contrib = tl.tile([P, TGRP * 256], dt.bfloat16, tag="contrib")
                dxv = dx[:, g0 * 16:(g0 + ng) * 16]
                dyv = dy[:, g0 * 16:(g0 + ng) * 16]
                nc.vector.tensor_tensor(
                    out=contrib[:, :ng * 256],
                    in0=_dc.replace(dxv, ap=[dxv.ap[0], [16, ng], [1, 16],
                                             [0, 16]]),
                    in1=_dc.replace(dyv, ap=[dyv.ap[0], [16, ng], [0, 16],
                                             [1, 16]]),
                    op=mybir.AluOpType.mult)
                btg = tl.tile([P, TGRP * 64], dt.bfloat16, tag="btg")
                btv = btl_sb[:, g0:g0 + ng]
                nc.vector.tensor_tensor(
                    out=btg[:, :ng * 64],
                    in0=_dc.replace(btv, ap=[btv.ap[0], [btv.ap[1][0], ng],
                                             [0, 64]]),
                    in1=_dc.replace(iota64[:], ap=[iota64[:].ap[0], [0, ng],
                                                   iota64[:].ap[1]]),
                    op=mybir.AluOpType.is_equal)
                for j in range(ng):
                    t = g0 + j
                    nc.tensor.matmul(G_ps[:], lhsT=btg[:, j * 64:(j + 1) * 64],
                                     rhs=contrib[:, j * 256:(j + 1) * 256],
                                     start=(t == 0), stop=(t == NT - 1),
                                     skip_group_check=True)
            G_sb = cst.tile([64, 256], dt.float32)
            nc.scalar.activation(G_sb[:], G_ps[:], AF.Identity)
            nc.sync.dma_start(g_out[:], G_sb[:])
    nc.compile()
    return nc


def kernel(**inputs):
    global _T, _OFF, _GROUPS, _GTMAX, _TOT
    global LAST_EXEC_NS, LAST_INSTS
    from concourse.bass_utils import run_bass_kernel_spmd

    x = np.asarray(inputs["x"], np.float32)
    prep = _host_prep(x, np.asarray(inputs["edge_index"]),
                      np.asarray(inputs["batch"]))
    T, off, TOT = prep["T"], prep["off"], prep["TOT"]

    groups = []
    for hh in range(2):
        gl = []
        t0 = 0
        while t0 < NT:
            tiles = list(range(t0, min(t0 + GRP, NT)))
            soff = int(off[tiles[0], hh])
            Tg = int(sum(T[t, hh] for t in tiles))
            gl.append((tiles, soff, Tg))
            t0 += GRP
        groups.append(gl)
    _T, _OFF, _GROUPS, _TOT = T, off, groups, TOT
    _GTMAX = max(Tg for gl in groups for (_, _, Tg) in gl)

    ck = (TOT, tuple(T.reshape(-1).tolist()))
    if ck not in _CACHE:
        _CACHE[ck] = _build()
    nc = _CACHE[ck]

    S = np.asarray(inputs["S"], np.float32).reshape(256, 384)
    SfT = S.T.copy()
    srow = (-0.5 * (S * S).sum(axis=1)).reshape(1, 256).astype(np.float32)
    in_maps = []
    for c in range(NC_):
        m = dict(
            xTt=prep["xTt"], idx16=prep["idx16"][c], ohw16=prep["ohw"][c],
            batch16=prep["batch16"][c],
            SfT=SfT.astype(ml_dtypes.bfloat16),
            srow=srow,
            W1=np.asarray(inputs["W1"], np.float32).astype(ml_dtypes.bfloat16),
            W2=np.asarray(inputs["W2"], np.float32).astype(ml_dtypes.bfloat16),
            W3=np.asarray(inputs["W3"], np.float32).astype(ml_dtypes.bfloat16),
            b1=np.asarray(inputs["b1"], np.float32).reshape(C, 1),
            b2=np.asarray(inputs["b2"], np.float32).reshape(C, 1),
            b3=np.asarray(inputs["b3"], np.float32).reshape(C, 1),
        )
        in_maps.append(m)
    kw = {}
    if TRACE:
        kw = dict(trace=True)
    res = run_bass_kernel_spmd(nc, in_maps, core_ids=list(range(NC_)), **kw)
    LAST_EXEC_NS = res.exec_time_ns
    LAST_INSTS = res.instructions_and_trace[0] if res.instructions_and_trace else None
    G = np.zeros((64, 256), np.float64)
    for c in range(NC_):
        G += res.results[c]["g_out"].astype(np.float64)
    lin_W = np.asarray(inputs["lin_W"], np.float32)
    lin_b = np.asarray(inputs["lin_b"], np.float32)
    z = G.astype(np.float32) @ lin_W.T + lin_b
    return (1.0 / (1.0 + np.exp(-z))).astype(np.float32)
